# revision 1
# baseline (speedup 1.0000x reference)
"""Trainium2 Bass kernel for the DPAAUser3D segment-reduce problem.

Computes, for x[B=2,C=8,D=H=W=128] and attentions[B,C,512,1]:
  onehot = one_hot(argmax_c x)                      (per-voxel channel argmax)
  adj    = avgpool_8x8x8(onehot)                    ([B,C,16,16,16], = counts/512)
  corr[b,c,D,H,W] = att[b,c,(D//16*8+H//16)*8+W//16] * adj[b,c,D%16,H%16,W%16]
  out1   = x * (1+corr)^2
  out2   = corr

Sharding: data-parallel over the D axis (16 slices per core, 8 cores). The
argmax and pooling blocks are D-local, so each core computes its two pooled
kd-blocks exactly; one 16KB AllGather per batch element distributes the full
pooled count map to every core for the correction phase.

Phase 1 processes (b,d) slabs with H on partitions (needed by the pooling
matmul which contracts over H). Phase 2 re-reads x with partitions mapped to
(kd, H//16) so every DMA (x in, out1/out2 out) runs in contiguous 8KB bursts.
"""

import sys

import numpy as np

try:
    import concourse.bass as bass
except ImportError:  # fresh grading dir: concourse lives in the repo checkout
    for p in ("/opt/trn_rl_repo", "/root/.axon_site/_ro/trn_rl_repo"):
        if p not in sys.path:
            sys.path.insert(0, p)
    import concourse.bass as bass

import ml_dtypes
import concourse.bacc as bacc
import concourse.mybir as mybir
import concourse.tile as tile
from concourse.tile import add_dep_helper
from concourse import bass_utils

B, C, D, H, W = 2, 8, 128, 128, 128
POOL = 8          # pooling block edge
PATCH = 16        # fold patch edge
G = D // PATCH    # 8 patches per spatial dim
NCORES = 8
DL = D // NCORES  # 16 d-slices per core
PD = DL // POOL   # 2 pooled kd-blocks per core

F32 = mybir.dt.float32
BF16 = mybir.dt.bfloat16

_CACHE = {}


def _build_nc():
    nc = bacc.Bacc("TRN2", target_bir_lowering=False, debug=False,
                   num_devices=NCORES)

    xs = nc.dram_tensor("xs", [B, C, DL, H, W], F32, kind="ExternalInput").ap()
    # arep[b,c,q,wp] = att[b,c, core*64 + (q%8)*8 + wp] / 512  (q = kd*8+hp)
    arep = nc.dram_tensor("arep", [B, C, 128, G], F32, kind="ExternalInput").ap()
    pmat = nc.dram_tensor("pmat", [H, PATCH], BF16, kind="ExternalInput").ap()
    o1 = nc.dram_tensor("o1", [B, C, DL, H, W], F32, kind="ExternalOutput").ap()
    o2 = nc.dram_tensor("o2", [B, C, DL, H, W], F32, kind="ExternalOutput").ap()

    FS = C * PATCH * PATCH  # 2048: per-kd free size of the pooled-count map

    with tile.TileContext(nc) as tc:
        with (
            tc.tile_pool(name="big", bufs=1) as big,
            tc.tile_pool(name="p1", bufs=3) as p1,
            tc.tile_pool(name="p2", bufs=3) as p2,
            tc.tile_pool(name="psum", bufs=1, space="PSUM") as pp,
            tc.tile_pool(name="dram", bufs=1, space="DRAM") as dram,
        ):
            Pm = big.tile([128, PATCH], BF16, name="Pm")
            Ar = big.tile([128, B, C, G], F32, name="Ar")
            # AdjR[q, b, (c,kh,kw)]: pooled counts, kd=q//8 replicated over hp
            AdjR = big.tile([128, B, FS], F32, name="AdjR")

            nc.sync.dma_start(out=Pm, in_=pmat)
            for b in range(B):
                nc.sync.dma_start(out=Ar[:, b], in_=arep[b].transpose([1, 0, 2]))

            psums = {}
            for b in range(B):
                for pd in range(PD):
                    for hf in range(2):
                        t = pp.tile([16, 512], F32, name=f"ps{b}{pd}{hf}",
                                    tag=f"ps{b}{pd}{hf}")
                        psums[(b, pd, hf)] = t

            adj_in = [dram.tile([PD, C, 16, 16], F32, name=f"adj_in{b}")
                      for b in range(B)]
            adj_gat = [dram.tile([NCORES, PD, C, 16, 16], F32,
                                 name=f"adj_gat{b}", addr_space="Shared")
                       for b in range(B)]

            # ---- phase 1: argmax one-hot + pooled counts ----
            last_p1_dve = None
            last_slab_load = None
            for b in range(B):
                for d in range(DL):
                    slab = p1.tile([128, C, W], F32, name="slab", tag="slab")
                    last_slab_load = nc.sync.dma_start(
                        out=slab, in_=xs[b, :, d].transpose([1, 0, 2]))
                    t1 = p1.tile([128, 4, W], F32, name="t1", tag="t1")
                    nc.vector.tensor_max(t1, slab[:, 0:4, :], slab[:, 4:8, :])
                    t2 = p1.tile([128, 2, W], F32, name="t2", tag="t2")
                    nc.vector.tensor_max(t2, t1[:, 0:2, :], t1[:, 2:4, :])
                    M = p1.tile([128, W], F32, name="M", tag="M")
                    nc.vector.tensor_max(M, t2[:, 0, :], t2[:, 1, :])
                    eq = p1.tile([128, C, W], BF16, name="eq", tag="eq")
                    nc.vector.tensor_tensor(
                        eq, slab, M.unsqueeze(1).broadcast_to([128, C, W]),
                        op=mybir.AluOpType.is_equal)
                    eqf = eq.rearrange("p c w -> p (c w)")
                    pd, dd = d // POOL, d % POOL
                    for hf in range(2):
                        nc.tensor.matmul(psums[(b, pd, hf)], lhsT=Pm,
                                         rhs=eqf[:, hf * 512:(hf + 1) * 512],
                                         start=(dd == 0), stop=(dd == POOL - 1))
                    if dd == POOL - 1:
                        adjp = p1.tile([16, C, 16], F32, name="adjp", tag="adjp")
                        for hf in range(2):
                            src = psums[(b, pd, hf)].rearrange(
                                "p (c wb wi) -> p c wb wi", c=4, wb=16, wi=8)
                            last_p1_dve = nc.vector.reduce_sum(
                                adjp[:, hf * 4:(hf + 1) * 4, :], src,
                                axis=mybir.AxisListType.X)
                        # payload [pd][c, ph, pw]; on the scalar ring (idle
                        # until phase 2) so neither the sync ring nor the
                        # gpsimd collective stream stalls behind this DMA's
                        # DVE-reduce dependency
                        nc.scalar.dma_start(out=adj_in[b][pd].transpose([1, 0, 2]),
                                            in_=adjp)
                # per-b AllGather: fires mid-kernel, overlaps remaining work
                nc.gpsimd.collective_compute(
                    "AllGather", mybir.AluOpType.bypass,
                    replica_groups=[list(range(NCORES))],
                    ins=[adj_in[b].opt()], outs=[adj_gat[b].opt()])
                # gathered [core,pd,c,ph,pw] flat == [kd, (c,kh,kw)]; load with
                # 8x partition replication: q = kd*8 + hp reads row kd = q//8.
                # On the gpsimd stream, which is already blocked on this
                # AllGather; sync/scalar rings keep flowing.
                rep = bass.AP(tensor=adj_gat[b].tensor, offset=adj_gat[b].offset,
                              ap=[[FS, DL], [0, POOL], [1, FS]])
                nc.gpsimd.dma_start(out=AdjR[:, b], in_=rep)

            # ---- phase 2: correction + outputs (partitions = (kd, hp)) ----
            for b in range(B):
                for c in range(C):
                    xv = xs[b, c].rearrange("d (a k) w -> (d a) (k w)", a=POOL)
                    x2 = p2.tile([128, PATCH * W], F32, name="x2", tag="x2",
                                 bufs=4)
                    x2_ld = nc.sync.dma_start(out=x2, in_=xv)
                    # keep the sync ring draining phase-1 slab loads first
                    add_dep_helper(x2_ld.ins, last_slab_load.ins, False,
                                   "phase-1 loads first")
                    corr = p2.tile([128, PATCH, G, PATCH], F32, name="corr",
                                   tag="corr")
                    a_b = Ar[:, b, c].unsqueeze(1).unsqueeze(3).broadcast_to(
                        [128, PATCH, G, PATCH])
                    r_b = AdjR[:, b].rearrange(
                        "p (c kh kw) -> p c kh kw", c=C, kh=PATCH)[:, c] \
                        .unsqueeze(2).broadcast_to([128, PATCH, G, PATCH])
                    corr_i = nc.vector.tensor_mul(corr, a_b, r_b)
                    # DVE must finish all phase-1 work before phase-2; without
                    # this the scheduler can park DVE on corr (blocked on the
                    # AllGather) while ready phase-1 slabs starve behind it
                    add_dep_helper(corr_i.ins, last_p1_dve.ins, False,
                                   "phase-1 DVE first")
                    corr_f = corr.rearrange("p a g k -> p (a g k)")
                    u2 = p2.tile([128, PATCH * W], F32, name="u2", tag="u2",
                                 bufs=3)
                    nc.scalar.activation(u2, corr_f,
                                         mybir.ActivationFunctionType.Square,
                                         bias=1.0, scale=1.0)
                    o1t = p2.tile([128, PATCH * W], F32, name="o1t", tag="o1t",
                                  bufs=3)
                    nc.vector.tensor_mul(o1t, x2, u2)
                    ov1 = o1[b, c].rearrange("d (a k) w -> (d a) (k w)", a=POOL)
                    ov2 = o2[b, c].rearrange("d (a k) w -> (d a) (k w)", a=POOL)
                    nc.scalar.dma_start(out=ov2, in_=corr_f)
                    nc.sync.dma_start(out=ov1, in_=o1t)

    nc.compile()
    return nc


def _fix_ties(x):
    """The device one-hot marks every channel equal to the max; the reference
    one_hot(argmax) marks only the first. Nudge later tied channels down by
    one ulp so a plain equality compare reproduces first-match semantics
    (out1 changes by <=1 ulp at those voxels)."""
    mx = x.max(axis=1, keepdims=True)
    ties = x == mx
    multi = ties.sum(axis=1) > 1
    if not multi.any():
        return x
    x = x.copy()
    for b, d, h, w in np.argwhere(multi):
        cs = np.flatnonzero(ties[b, :, d, h, w])
        for c in cs[1:]:
            x[b, c, d, h, w] = np.nextafter(x[b, c, d, h, w], -np.inf)
    return x


def _host_inputs(x, attentions):
    """Build per-core input maps from full inputs."""
    x = _fix_ties(x)
    att = attentions[..., 0].astype(np.float32) * np.float32(1.0 / 512.0)
    att_p = att.reshape(B, C, G, G, G)  # [b, c, dp, hp, wp]
    pm = np.zeros((H, PATCH), dtype=ml_dtypes.bfloat16)
    pm[np.arange(H), np.arange(H) // POOL] = 1.0

    in_maps = []
    for core in range(NCORES):
        xs = np.ascontiguousarray(x[:, :, core * DL:(core + 1) * DL])
        # arep[b,c,q,wp] = att_p[b,c,core, q%8, wp]  (q = kd*8 + hp)
        arep = np.ascontiguousarray(
            np.tile(att_p[:, :, core], (1, 1, DL, 1)).reshape(B, C, 128, G))
        in_maps.append({"xs": xs, "arep": arep, "pmat": pm})
    return in_maps


def kernel(x, attentions):
    x = np.asarray(x, dtype=np.float32)
    attentions = np.asarray(attentions, dtype=np.float32)

    if "nc" not in _CACHE:
        _CACHE["nc"] = _build_nc()
    nc = _CACHE["nc"]

    in_maps = _host_inputs(x, attentions)
    res = bass_utils.run_bass_kernel_spmd(nc, in_maps,
                                          core_ids=list(range(NCORES)))

    out1 = np.empty((B, C, D, H, W), np.float32)
    out2 = np.empty((B, C, D, H, W), np.float32)
    for core in range(NCORES):
        out1[:, :, core * DL:(core + 1) * DL] = res.results[core]["o1"]
        out2[:, :, core * DL:(core + 1) * DL] = res.results[core]["o2"]
    return out1, out2



# revision 11
# speedup vs baseline: 1.5410x; 1.5410x over previous
"""Trainium2 Bass kernel for the DPAAUser3D segment-reduce problem.

Computes, for x[B=2,C=8,D=H=W=128] and attentions[B,C,512,1]:
  onehot = one_hot(argmax_c x)                      (per-voxel channel argmax)
  adj    = avgpool_8x8x8(onehot)                    ([B,C,16,16,16], = counts/512)
  corr[b,c,D,H,W] = att[b,c,(D//16*8+H//16)*8+W//16] * adj[b,c,D%16,H%16,W%16]
  out1   = x * (1+corr)^2
  out2   = corr

Single-pass design: x is loaded once per core (data-parallel over D, 16
d-slices each) and stays resident in SBUF between the counting phase and the
output phase. All bulk IO runs in fp16, cutting per-core HBM traffic to
~27 MiB (8.4 in + 16.8 out + small collective/replication traffic).

Per (b,c), x is one SBUF tile with partitions=(d_local, h//16) and
free=(h%16, w) — contiguous 4 KiB DMA lines both directions. In this layout:
  - channel argmax one-hot: tree max over the 8 c-tiles + is_equal
  - 8x8x8 pooled counts: matmul with a 0/1 selector (contracts the d-blocks
    on partitions) + strided reduce_sum over the free h/w sub-blocks
  - a per-batch 8 KiB AllGather distributes the pooled count map; corr is a
    broadcasted tensor_mul, (1+corr)^2 runs on the scalar engine.

fp16 correctness: the host computes the f32 argmax (reference semantics) and
nudges any non-argmax channel that collides with the channel max in fp16 down
one ulp, so the device equality compare reproduces one_hot(argmax(f32 x))
exactly and the pooled counts are exact integers (<=512, exact in fp16). The
remaining error is fp16 rounding on x and on each output store (~1e-3 rel).
"""

import sys

import numpy as np

try:
    import concourse.bass as bass
except ImportError:  # fresh grading dir: concourse lives in the repo checkout
    for p in ("/opt/trn_rl_repo", "/root/.axon_site/_ro/trn_rl_repo"):
        if p not in sys.path:
            sys.path.insert(0, p)
    import concourse.bass as bass

import concourse.bacc as bacc
import concourse.mybir as mybir
import concourse.tile as tile
from concourse import bass_utils

B, C, D, H, W = 2, 8, 128, 128, 128
POOL = 8          # pooling block edge
PATCH = 16        # fold patch edge
G = D // PATCH    # 8 patches per spatial dim
NCORES = 8
DL = D // NCORES  # 16 d-slices per core
PD = DL // POOL   # 2 pooled kd-blocks per core
FS = C * PATCH * PATCH  # 2048: free size of the per-batch pooled-count map

F32 = mybir.dt.float32
F16 = mybir.dt.float16

_CACHE = {}


def _build_nc():
    nc = bacc.Bacc("TRN2", target_bir_lowering=False, debug=False,
                   num_devices=NCORES)

    xs = nc.dram_tensor("xs", [B, C, DL, H, W], F16, kind="ExternalInput").ap()
    # arep[b,c,p,wp] = att[b,c, (core*8 + p%8)*8 + wp] / 512   (p = dl*8+hb)
    arep = nc.dram_tensor("arep", [B, C, 128, G], F32, kind="ExternalInput").ap()
    # sel[p, (p//64)*8 + p%8] = 1: contracts the two 8-d-slice blocks
    sel = nc.dram_tensor("sel", [128, 16], F16, kind="ExternalInput").ap()
    o1 = nc.dram_tensor("o1", [B, C, DL, H, W], F16, kind="ExternalOutput").ap()
    o2 = nc.dram_tensor("o2", [B, C, DL, H, W], F16, kind="ExternalOutput").ap()

    with tile.TileContext(nc) as tc:
        with (
            tc.tile_pool(name="big", bufs=1) as big,
            tc.tile_pool(name="mx", bufs=1) as mxp,
            tc.tile_pool(name="eqp", bufs=3) as eqp,
            tc.tile_pool(name="p2", bufs=3) as p2,
            tc.tile_pool(name="psum", bufs=2, space="PSUM") as pp,
            tc.tile_pool(name="dram", bufs=1, space="DRAM") as dram,
        ):
            Sel = big.tile([128, 16], F16, name="Sel")
            Ar = big.tile([128, B, C, G], F32, name="Ar")
            X = {(b, c): big.tile([128, PATCH * W], F16, name=f"x{b}{c}")
                 for b in range(B) for c in range(C)}
            # AdjR[b][p=(dl,rep8), (c,j,l)] = counts[b,c,row dl] (8x partition rep)
            AdjR = [big.tile([128, FS], F16, name=f"AdjR{b}") for b in range(B)]
            Cnt = [big.tile([16, 2, C, PATCH], F16, name=f"cnt{b}")
                   for b in range(B)]

            nc.sync.dma_start(out=Sel, in_=sel)
            for b in range(B):
                nc.sync.dma_start(out=Ar[:, b], in_=arep[b].transpose([1, 0, 2]))
            for b in range(B):
                for c in range(C):
                    xv = xs[b, c].rearrange("d (a k) w -> (d a) (k w)", a=POOL)
                    nc.sync.dma_start(out=X[b, c], in_=xv)

            # row layout (j, c, l): j = hb*2+j2 has uniform stride for the
            # corr-phase view and the count store collapses to 3 DMA dims
            adj_in = [dram.tile([PD, PATCH, C, PATCH], F16, name=f"adj_in{b}")
                      for b in range(B)]
            adj_gat = [dram.tile([NCORES, PD, PATCH, C, PATCH], F16,
                                 name=f"adj_gat{b}", addr_space="Shared")
                       for b in range(B)]

            def phase1(b):
                # channel max (tree over the 8 resident c-tiles)
                m1 = [mxp.tile([128, PATCH * W], F16, name=f"m1_{i}",
                               tag=f"m1_{i}") for i in range(4)]
                for i in range(4):
                    nc.vector.tensor_max(m1[i], X[b, 2 * i], X[b, 2 * i + 1])
                m2 = [mxp.tile([128, PATCH * W], F16, name=f"m2_{i}",
                               tag=f"m2_{i}") for i in range(2)]
                nc.vector.tensor_max(m2[0], m1[0], m1[1])
                nc.vector.tensor_max(m2[1], m1[2], m1[3])
                M = mxp.tile([128, PATCH * W], F16, name="M", tag="M")
                nc.vector.tensor_max(M, m2[0], m2[1])
                for c in range(C):
                    eq = eqp.tile([128, PATCH * W], F16, name="eq", tag="eq")
                    nc.vector.tensor_tensor(eq, X[b, c], M,
                                            op=mybir.AluOpType.is_equal)
                    eqv = eq.rearrange("p (kc x) -> p kc x", kc=4)
                    for half in range(2):
                        ps = pp.tile([16, 512], F32, name="ps", tag=f"ps{half}")
                        nc.tensor.matmul(ps, lhsT=Sel, rhs=eqv[:, 2 * half],
                                         start=True, stop=False)
                        nc.tensor.matmul(ps, lhsT=Sel, rhs=eqv[:, 2 * half + 1],
                                         start=False, stop=True)
                        # ps free = (k4, l=16, w8): sum the 8x8 h/w sub-block
                        pv = ps.rearrange("p (k4 l w8) -> p l k4 w8",
                                          k4=4, l=PATCH, w8=POOL)
                        # counts are integers <=512: exact in fp16
                        with nc.allow_low_precision(reason="integer counts"):
                            nc.vector.reduce_sum(Cnt[b][:, half, c], pv,
                                                 axis=mybir.AxisListType.XY)
                # ship counts: partition (pd,hb) + free (j2,c,l) lands on the
                # contiguous (j=hb*2+j2, c, l) rows of adj_in
                cin = adj_in[b].rearrange("pd (hb j2) c l -> (pd hb) (j2 c l)",
                                          hb=POOL)
                nc.gpsimd.dma_start(
                    out=cin, in_=Cnt[b].rearrange("p j2 c l -> p (j2 c l)"))
                nc.gpsimd.collective_compute(
                    "AllGather", mybir.AluOpType.bypass,
                    replica_groups=[list(range(NCORES))],
                    ins=[adj_in[b].opt()], outs=[adj_gat[b].opt()])
                # gathered [kd, c, j, l]; load with 8x partition replication so
                # partition p=(dl,hb) holds row kd=dl
                rep = bass.AP(tensor=adj_gat[b].tensor,
                              offset=adj_gat[b].offset,
                              ap=[[FS, DL], [0, POOL], [1, FS]])
                nc.gpsimd.dma_start(out=AdjR[b], in_=rep)

            def phase2(b):
                for c in range(C):
                    corr = p2.tile([128, PATCH, G, PATCH], F16, name="corr",
                                   tag="corr")
                    a_b = Ar[:, b, c].unsqueeze(1).unsqueeze(3).broadcast_to(
                        [128, PATCH, G, PATCH])
                    r_b = AdjR[b].rearrange("p (kh c kw) -> p kh c kw",
                                            c=C, kh=PATCH)[:, :, c] \
                        .unsqueeze(2).broadcast_to([128, PATCH, G, PATCH])
                    nc.vector.tensor_mul(corr, a_b, r_b)
                    corr_f = corr.rearrange("p a g k -> p (a g k)")
                    ov2 = o2[b, c].rearrange("d (a k) w -> (d a) (k w)", a=POOL)
                    nc.gpsimd.dma_start(out=ov2, in_=corr_f)
                    u2 = p2.tile([128, PATCH * W], F16, name="u2", tag="u2")
                    nc.scalar.activation(u2, corr_f,
                                         mybir.ActivationFunctionType.Square,
                                         bias=1.0, scale=1.0)
                    o1t = p2.tile([128, PATCH * W], F16, name="o1t", tag="o1t")
                    nc.vector.tensor_mul(o1t, X[b, c], u2)
                    ov1 = o1[b, c].rearrange("d (a k) w -> (d a) (k w)", a=POOL)
                    nc.scalar.dma_start(out=ov1, in_=o1t)

            phase1(0)
            phase2(0)
            phase1(1)
            phase2(1)

    nc.compile()
    return nc


def _host_inputs(x, attentions):
    """Build per-core input maps from full f32 inputs (fp16 cast + argmax fix)."""
    am = np.argmax(x, axis=1)              # [B,D,H,W], first-max == reference
    xh = x.astype(np.float16)
    mx = xh.max(axis=1, keepdims=True)
    notam = (np.arange(C)[None, :, None, None, None] != am[:, None])
    coll = (xh == mx) & notam
    if coll.any():
        u = xh.view(np.uint16)
        pos = xh > 0
        zero = xh == 0
        down = np.where(pos, u - np.uint16(1),
                        np.where(zero, np.uint16(0x8001), u + np.uint16(1)))
        xh = np.where(coll, down.view(np.float16), xh)

    att = attentions[..., 0].astype(np.float32) * np.float32(1.0 / 512.0)
    att_p = att.reshape(B, C, G, G, G)     # [b, c, dp, hp, wp]
    selm = np.zeros((128, 16), np.float16)
    p = np.arange(128)
    selm[p, (p // 64) * 8 + (p % 8)] = 1.0

    in_maps = []
    for core in range(NCORES):
        xsc = np.ascontiguousarray(xh[:, :, core * DL:(core + 1) * DL])
        arep = np.ascontiguousarray(
            np.tile(att_p[:, :, core], (1, 1, DL, 1)).reshape(B, C, 128, G))
        in_maps.append({"xs": xsc, "arep": arep, "sel": selm})
    return in_maps


def kernel(x, attentions):
    x = np.asarray(x, dtype=np.float32)
    attentions = np.asarray(attentions, dtype=np.float32)

    if "nc" not in _CACHE:
        _CACHE["nc"] = _build_nc()
    nc = _CACHE["nc"]

    in_maps = _host_inputs(x, attentions)
    res = bass_utils.run_bass_kernel_spmd(nc, in_maps,
                                          core_ids=list(range(NCORES)))

    out1 = np.empty((B, C, D, H, W), np.float32)
    out2 = np.empty((B, C, D, H, W), np.float32)
    for core in range(NCORES):
        out1[:, :, core * DL:(core + 1) * DL] = res.results[core]["o1"]
        out2[:, :, core * DL:(core + 1) * DL] = res.results[core]["o2"]
    return out1, out2
